# revision 1
# baseline (speedup 1.0000x reference)
"""Trainium2 Bass kernel for nn_CrossAttentionBlock.

Reference computation (per batch b):
    q = x1 @ wq_w.T + wq_b              [n1, HD]   HD = 8 heads x 128
    k = x2 @ wk_w.T + wk_b              [n2, HD]
    v = x2 @ wv_w.T + wv_b              [n2, HD]
    scores_h = q_h @ k_h.T / sqrt(128) + B          [n1, n2] per head
    attn = softmax(scores, axis=-1)
    out_h = attn_h @ v_h                            [n1, 128]
    out = concat_h(out_h) @ proj_w.T + proj_b       [n1, 128]

Sharding: data-parallel over batch, 2 batches per core on 8 cores.

Kernel layout strategy (per core):
  - Everything transposed so the softmax-contraction dim (n2) lives on
    SBUF partitions:  S.T[n2, n1] = K @ Q.T  per head.
  - softmax without max-subtraction (scores are O(+-10), exp is safe in
    fp32/bf16) and with exp(B) folded multiplicatively:
        P.T = exp(S.T/sdk) * exp(B.T)
  - row sums l[n1] via ones-vector matmul over P.T (accumulated in PSUM).
  - out_h.T[d, n1] = sum_n2 V[n2,d].T-slices @ P.T   (V used in natural
    [n2, hd] layout as the stationary operand).
  - normalization deferred: out_h.T * (1/l) with 1/l replicated across
    partitions via a rank-1 ones matmul + approx reciprocal on DVE.
  - proj accumulated head-by-head into F.T[o, n1] (PSUM->SBUF adds),
    proj_b added as a rank-1 matmul, final PE transpose back to [n1, o].
"""

import sys

sys.path.insert(0, "/opt/trn_rl_repo")

import numpy as np

import concourse.bass as bass
import concourse.tile as tile
from concourse import mybir
from concourse.masks import make_identity

# ---------------------------------------------------------------------------
# Problem constants (hardcoded per contest rules; kernel.py is self-contained)
# ---------------------------------------------------------------------------
NUM_HEAD = 8
HIDDEN = 128  # head dim and final output dim
INPUT_DIM = 256
N1 = 1024
N2 = 1024
BATCH = 16
N_CORES = 8
BPC = BATCH // N_CORES  # batches per core
HD = NUM_HEAD * HIDDEN  # 1024
SDK = float(np.sqrt(np.float32(HIDDEN)))

F32 = mybir.dt.float32
F32R = mybir.dt.float32r
BF16 = mybir.dt.bfloat16
AF = mybir.ActivationFunctionType


# ---------------------------------------------------------------------------
# Post-pass: split multi-wait instructions into single-wait NOP prefixes.
# Walrus codegen in this container rejects instructions whose ISA struct has
# room for only one sync-wait command. A NoOp on the same engine queue
# carrying the extra waits is semantically identical (the sequencer executes
# waits in queue order before dispatching later instructions).
# ---------------------------------------------------------------------------
_ws_counter = [0]


def split_multi_waits(nc, cap=1):
    total = 0
    for fn in nc.m.functions:
        for blk in fn.blocks:
            insts = blk.instructions
            new = []
            changed = False
            for inst in insts:
                si = getattr(inst, "sync_info", None)
                waits = list(si.on_wait) if si is not None else []
                if len(waits) > cap:
                    for w in waits[:-cap]:
                        nop = mybir.InstNoOp(
                            name=f"I-wsplit-{_ws_counter[0]}", ins=[], outs=[]
                        )
                        _ws_counter[0] += 1
                        nop.engine = inst.engine
                        nop.sync_info = mybir.SyncInfo(on_wait=[w], on_update=[])
                        new.append(nop)
                        total += 1
                    inst.sync_info = mybir.SyncInfo(
                        on_wait=waits[-cap:], on_update=list(si.on_update)
                    )
                    changed = True
                new.append(inst)
            if changed:
                insts[:] = new
    return total


def _r(ap):
    """fp32 -> fp32r view for full-rate PE matmuls."""
    return ap.bitcast(F32R)


def build_bass(waitsplit=True, n_batches=BPC, n_heads=NUM_HEAD, do_attn=True):
    nc = bass.Bass()

    x1_d = nc.dram_tensor("x1", [BPC, N1, INPUT_DIM], F32, kind="ExternalInput")
    x2_d = nc.dram_tensor("x2", [BPC, N2, INPUT_DIM], F32, kind="ExternalInput")
    b_d = nc.dram_tensor("B", [N1, N2], F32, kind="ExternalInput")
    wq_d = nc.dram_tensor("wq_w", [HD, INPUT_DIM], F32, kind="ExternalInput")
    wk_d = nc.dram_tensor("wk_w", [HD, INPUT_DIM], F32, kind="ExternalInput")
    wv_d = nc.dram_tensor("wv_w", [HD, INPUT_DIM], F32, kind="ExternalInput")
    qb_d = nc.dram_tensor("wq_b", [HD], F32, kind="ExternalInput")
    kb_d = nc.dram_tensor("wk_b", [HD], F32, kind="ExternalInput")
    vb_d = nc.dram_tensor("wv_b", [HD], F32, kind="ExternalInput")
    pw_d = nc.dram_tensor("proj_w", [HIDDEN, HD], F32, kind="ExternalInput")
    pb_d = nc.dram_tensor("proj_b", [HIDDEN], F32, kind="ExternalInput")
    out_d = nc.dram_tensor("out", [BPC, N1, HIDDEN], F32, kind="ExternalOutput")

    NT1 = N1 // 128  # 8 n1 tiles
    NT2 = N2 // 128  # 8 n2 tiles
    CT = INPUT_DIM // 128  # 2 c tiles

    with tile.TileContext(nc) as tc:
        with (
            tc.tile_pool(name="const", bufs=1) as const,
            tc.tile_pool(name="psS", bufs=2, space="PSUM") as psS,
            tc.tile_pool(name="psO", bufs=1, space="PSUM") as psO,
            tc.tile_pool(name="psL", bufs=1, space="PSUM") as psL,
        ):
            ident = const.tile([128, 128], F32)
            make_identity(nc, ident)

            # weight/bias/B staging + one-time transposes
            wqT = const.tile([128, CT, HD], F32R)  # wq_w.T  [c, hd]
            wkT = const.tile([128, CT, HD], F32R)
            wvT = const.tile([128, CT, HD], F32R)
            projT = const.tile([128, NUM_HEAD, HIDDEN], F32R)  # proj_w.T [hd, o]
            eb = const.tile([128, NT2, N1], BF16)  # exp(B.T)  [n2, n1]
            qb_sb = const.tile([128, NUM_HEAD], F32)
            kb_sb = const.tile([128, NUM_HEAD], F32)
            vb_row = const.tile([1, HD], F32R)
            pb_row = const.tile([1, HIDDEN], F32R)
            ones_col = const.tile([1, 128], F32R)
            ones_row = const.tile([1, 512], F32R)
            ones_bf = const.tile([128, 1], BF16)

            ones_st = const.tile([1, 512], F32)
            nc.vector.memset(ones_st, 1.0)
            nc.vector.tensor_copy(ones_col, ones_st[:, :128])
            nc.vector.tensor_copy(ones_row, ones_st)
            nc.vector.memset(ones_bf, 1.0)
            nc.sync.dma_start(out=qb_sb, in_=qb_d.rearrange("(t p) -> p t", p=128))
            nc.sync.dma_start(out=kb_sb, in_=kb_d.rearrange("(t p) -> p t", p=128))
            bias_stage = const.tile([1, HD + HIDDEN], F32)
            nc.sync.dma_start(
                out=bias_stage[:, :HD], in_=vb_d.rearrange("(a n) -> a n", a=1)
            )
            nc.sync.dma_start(
                out=bias_stage[:, HD:], in_=pb_d.rearrange("(a n) -> a n", a=1)
            )
            nc.vector.tensor_copy(vb_row, bias_stage[:, :HD])
            nc.vector.tensor_copy(pb_row, bias_stage[:, HD:])

            with tc.tile_pool(name="stage", bufs=1) as stage:
                # --- weights: [hd, c] -> [c, hd] via PE transposes
                for w_d, wT in ((wq_d, wqT), (wk_d, wkT), (wv_d, wvT)):
                    wst = stage.tile([128, HD // 128, INPUT_DIM], F32, tag="wst")
                    nc.sync.dma_start(
                        out=wst, in_=w_d.rearrange("(t p) c -> p t c", p=128)
                    )
                    for ct in range(CT):
                        ps = psS.tile([128, 1024], F32, tag="s")
                        for t in range(HD // 128):
                            nc.tensor.transpose(
                                ps[:, t * 128 : (t + 1) * 128],
                                wst[:, t, ct * 128 : (ct + 1) * 128],
                                ident,
                            )
                        nc.scalar.copy(wT[:, ct, :], ps)
                # --- proj_w [o=128, hd] -> projT [hd, o]
                pwst = stage.tile([128, HD], F32)
                nc.sync.dma_start(out=pwst, in_=pw_d[:, :])
                ps = psS.tile([128, 1024], F32, tag="s")
                for h in range(NUM_HEAD):
                    nc.tensor.transpose(
                        ps[:, h * 128 : (h + 1) * 128],
                        pwst[:, h * 128 : (h + 1) * 128],
                        ident,
                    )
                nc.scalar.copy(projT.rearrange("p h o -> p (h o)"), ps)
                # --- B [n1, n2] -> eb = exp(B.T) [n2, n1] (bf16)
                bst = stage.tile([128, NT1, N2], F32)
                nc.sync.dma_start(
                    out=bst, in_=b_d.rearrange("(t p) m -> p t m", p=128)
                )
                for n2t in range(NT2):
                    ps = psS.tile([128, 1024], F32, tag="s")
                    for n1t in range(NT1):
                        nc.tensor.transpose(
                            ps[:, n1t * 128 : (n1t + 1) * 128],
                            bst[:, n1t, n2t * 128 : (n2t + 1) * 128],
                            ident,
                        )
                    nc.scalar.activation(eb[:, n2t, :], ps, AF.Exp)

            with (
                tc.tile_pool(name="xin", bufs=1) as xin,
                tc.tile_pool(name="qkv", bufs=1) as qkv,
                tc.tile_pool(name="attn", bufs=4) as attn,
                tc.tile_pool(name="head", bufs=2) as headp,
                tc.tile_pool(name="proj", bufs=2) as projp,
            ):
                for b in range(n_batches):
                    # ---------------- input transposes ----------------
                    x1T = xin.tile([128, CT, N1], F32R, tag="x1T")
                    x2T = xin.tile([128, CT, N2], F32R, tag="x2T")
                    for x_d, xT, nt in ((x1_d, x1T, NT1), (x2_d, x2T, NT2)):
                        xst = xin.tile([128, nt, INPUT_DIM], F32, tag="xst")
                        nc.sync.dma_start(
                            out=xst, in_=x_d[b].rearrange("(t p) c -> p t c", p=128)
                        )
                        for ct in range(CT):
                            ps = psS.tile([128, 1024], F32, tag="s")
                            for t in range(nt):
                                nc.tensor.transpose(
                                    ps[:, t * 128 : (t + 1) * 128],
                                    xst[:, t, ct * 128 : (ct + 1) * 128],
                                    ident,
                                )
                            nc.vector.tensor_copy(xT[:, ct, :], ps)

                    # ---------------- QKV projections ----------------
                    qT = qkv.tile([128, NUM_HEAD, N1], BF16, tag="qT")  # [d, n1]/head
                    kT = qkv.tile([128, NUM_HEAD, N2], BF16, tag="kT")  # [d, n2]/head
                    vN = qkv.tile([128, NT2, HD], BF16, tag="vN")  # [n2, hd]
                    # Q.T / K.T : out[hd_tile, n] ; lhsT = w.T slice, rhs = x.T
                    for xT, wT, dstT, bias, n in (
                        (x1T, wqT, qT, qb_sb, N1),
                        (x2T, wkT, kT, kb_sb, N2),
                    ):
                        for h in range(NUM_HEAD):
                            ps = psS.tile([128, 1024], F32, tag="s")
                            for half in range(2):
                                sl = slice(half * 512, half * 512 + 512)
                                for ct in range(CT):
                                    nc.tensor.matmul(
                                        ps[:, sl],
                                        wT[:, ct, h * 128 : (h + 1) * 128],
                                        xT[:, ct, sl],
                                        start=(ct == 0),
                                        stop=(ct == CT - 1),
                                    )
                            nc.scalar.activation(
                                dstT[:, h, :], ps, AF.Identity,
                                bias=bias[:, h : h + 1],
                            )
                    # V natural: out[n2_tile, hd] ; lhsT = x2.T slice, rhs = wv.T
                    for t in range(NT2):
                        ps = psS.tile([128, 1024], F32, tag="s")
                        for half in range(2):
                            sl = slice(half * 512, half * 512 + 512)
                            for ct in range(CT):
                                nc.tensor.matmul(
                                    ps[:, sl],
                                    x2T[:, ct, t * 128 : (t + 1) * 128],
                                    wvT[:, ct, sl],
                                    start=(ct == 0),
                                    stop=False,
                                )
                            # rank-1 bias: ones[n2] x wv_b[hd]
                            nc.tensor.matmul(
                                ps[:, sl],
                                ones_col,
                                vb_row[:, sl],
                                start=False,
                                stop=True,
                            )
                        nc.vector.tensor_copy(vN[:, t, :], ps)

                    # ---------------- attention + proj ----------------
                    # Each head's epilogue (normalize by 1/l, project into
                    # F.T) is deferred and emitted interleaved with the NEXT
                    # head's tile loop so the in-order PE queue never stalls
                    # on the DVE epilogue chain.
                    ftacc = projp.tile([128, N1], F32, tag="ft")  # F.T accum [o, n1]

                    def epi_a(st):  # row-sum copy + replicate matmul
                        st["l_sb"] = headp.tile([1, N1], F32R, tag="lsb", name="l_sb")
                        nc.vector.tensor_copy(st["l_sb"], st["pl"])
                        st["lrep"] = psS.tile([128, 1024], F32, tag="s", name="lrep")
                        for half in range(2):
                            sl = slice(half * 512, half * 512 + 512)
                            nc.tensor.matmul(
                                st["lrep"][:, sl], ones_col, st["l_sb"][:, sl],
                                start=True, stop=True,
                            )

                    def epi_b1(st):
                        st["linv"] = headp.tile([128, N1], F32, tag="linv", name="linv")
                        nc.vector.reciprocal_approx_fast(st["linv"], st["lrep"])

                    def epi_b2(st):
                        st["outT"] = headp.tile([128, N1], F32R, tag="outT", name="outT")
                        nc.vector.tensor_mul(st["outT"], st["po"], st["linv"])

                    def epi_c(st):  # proj into F.T accumulation
                        h = st["h"]
                        fps = psS.tile([128, 1024], F32, tag="s")
                        for half in range(2):
                            sl = slice(half * 512, half * 512 + 512)
                            nc.tensor.matmul(
                                fps[:, sl], projT[:, h, :], st["outT"][:, sl],
                                start=True, stop=(h != 0),
                            )
                            if h == 0:
                                nc.tensor.matmul(
                                    fps[:, sl], pb_row, ones_row,
                                    start=False, stop=True,
                                )
                        if h == 0:
                            nc.vector.tensor_copy(ftacc, fps)
                        else:
                            nc.vector.tensor_add(ftacc, ftacc, fps)

                    pending = None
                    for h in range(n_heads if do_attn else 0):
                        po = psO.tile([128, N1], F32, tag="o")  # out_h.T accum
                        pl = psL.tile([1, N1], F32, tag="l")  # row sums
                        for n2t in range(NT2):
                            sps = psS.tile([128, 1024], F32, tag="s")
                            p_t = attn.tile([128, N1], BF16, tag="p")
                            for half in range(2):
                                sl = slice(half * 512, half * 512 + 512)
                                nc.tensor.matmul(
                                    sps[:, sl],
                                    kT[:, h, n2t * 128 : (n2t + 1) * 128],
                                    qT[:, h, sl],
                                    start=True,
                                    stop=True,
                                )
                            if n2t == 0 and pending:
                                epi_a(pending)
                            # P = exp(S/sdk) * exp(B.T)
                            nc.scalar.activation(p_t, sps, AF.Exp, scale=1.0 / SDK)
                            nc.vector.tensor_mul(p_t, p_t, eb[:, n2t, :])
                            if pending:
                                if n2t == 0:
                                    epi_b1(pending)
                                elif n2t == 1:
                                    epi_b2(pending)
                                elif n2t == 2:
                                    epi_c(pending)
                                    pending = None
                            first, last = n2t == 0, n2t == NT2 - 1
                            for half in range(2):
                                sl = slice(half * 512, half * 512 + 512)
                                nc.tensor.matmul(
                                    po[:, sl],
                                    vN[:, n2t, h * 128 : (h + 1) * 128],
                                    p_t[:, sl],
                                    start=first,
                                    stop=last,
                                    skip_group_check=True,
                                )
                                nc.tensor.matmul(
                                    pl[:, sl],
                                    ones_bf,
                                    p_t[:, sl],
                                    start=first,
                                    stop=last,
                                    skip_group_check=True,
                                )
                        pending = {"h": h, "po": po, "pl": pl}
                    if pending:
                        epi_a(pending)
                        epi_b1(pending)
                        epi_b2(pending)
                        epi_c(pending)
                        pending = None

                    # ---------------- final transpose + store ----------------
                    if not do_attn or n_heads < 1:
                        nc.vector.memset(ftacc, 0.0)
                    ofin = projp.tile([128, NT1, HIDDEN], F32, tag="ofin")
                    for t4 in range(0, NT1, 4):
                        ps = psS.tile([128, 1024], F32, tag="s")
                        for j in range(4):
                            t = t4 + j
                            nc.tensor.transpose(
                                ps[:, j * 128 : (j + 1) * 128],
                                ftacc[:, t * 128 : (t + 1) * 128],
                                ident,
                            )
                        nc.scalar.copy(
                            ofin[:, t4 : t4 + 4, :].rearrange("p t o -> p (t o)"),
                            ps[:, 0:512],
                        )
                    nc.sync.dma_start(
                        out=out_d[b].rearrange("(t p) o -> p t o", p=128), in_=ofin
                    )

    # Populate .instr bytes for extended-inst InstISA subclasses (the
    # custom-DVE reciprocal) — Tile/raw-Bass skips this Bacc.compile() pass.
    from concourse.library_overlay import lower_extended_insts

    lower_extended_insts(nc)
    if waitsplit:
        split_multi_waits(nc)
    return nc


_NC_CACHE = {}


def kernel(**inputs) -> np.ndarray:
    from concourse.bass_utils import run_bass_kernel_spmd

    x1 = np.ascontiguousarray(np.asarray(inputs["x1"], dtype=np.float32))
    x2 = np.ascontiguousarray(np.asarray(inputs["x2"], dtype=np.float32))
    shared = {
        n: np.ascontiguousarray(np.asarray(inputs[n], dtype=np.float32))
        for n in (
            "B", "wq_w", "wq_b", "wk_w", "wk_b", "wv_w", "wv_b", "proj_w", "proj_b"
        )
    }

    if "nc" not in _NC_CACHE:
        _NC_CACHE["nc"] = build_bass()
    nc = _NC_CACHE["nc"]

    in_maps = []
    for c in range(N_CORES):
        m = {"x1": x1[c * BPC : (c + 1) * BPC], "x2": x2[c * BPC : (c + 1) * BPC]}
        m.update(shared)
        in_maps.append(m)

    res = run_bass_kernel_spmd(nc, in_maps, core_ids=list(range(N_CORES)))
    out = np.concatenate([r["out"] for r in res.results], axis=0)
    return out



# revision 4
# speedup vs baseline: 1.3116x; 1.3116x over previous
"""Trainium2 Bass kernel for nn_CrossAttentionBlock.

Reference computation (per batch b):
    q = x1 @ wq_w.T + wq_b              [n1, HD]   HD = 8 heads x 128
    k = x2 @ wk_w.T + wk_b              [n2, HD]
    v = x2 @ wv_w.T + wv_b              [n2, HD]
    scores_h = q_h @ k_h.T / sqrt(128) + B          [n1, n2] per head
    attn = softmax(scores, axis=-1)
    out_h = attn_h @ v_h                            [n1, 128]
    out = concat_h(out_h) @ proj_w.T + proj_b       [n1, 128]

Sharding: data-parallel over batch, 2 batches per core on 8 cores.

Kernel layout strategy (per core):
  - Everything transposed so the softmax-contraction dim (n2) lives on
    SBUF partitions:  S.T[n2, n1] = K @ Q.T  per head.
  - softmax without max-subtraction (scores are O(+-10), exp is safe in
    fp32/bf16) and with exp(B) folded multiplicatively:
        P.T = exp(S.T/sdk) * exp(B.T)
  - row sums replicated across partitions in ONE matmul chain per head:
    all-ones [128,128] stationary over P.T tiles, PSUM-accumulated,
    directly yields lrep[o,n1] = l[n1] on every partition.
  - out_h.T[d, n1] = sum_n2 V[n2,d].T-slices @ P.T   (V used in natural
    [n2, hd] layout as the stationary operand).
  - normalization deferred: out_h.T * (1/lrep) via approx reciprocal.
  - proj accumulated head-by-head into F.T[o, n1], proj_b (with wv_b
    pre-folded on the host: attn rows sum to 1, so attn@1*vb.T = vb)
    added as a per-partition scalar add, final PE transpose to [n1, o].
  - All matmul operands bf16 (except f32 transposes and f32r outT for
    the proj moving operand): full PE rate at much lower power, which
    matters because the baseline showed ~110us of half-rate PE
    throttling.
"""

import sys

sys.path.insert(0, "/opt/trn_rl_repo")

import numpy as np

import concourse.bass as bass
import concourse.tile as tile
from concourse import mybir
from concourse.masks import make_identity

# ---------------------------------------------------------------------------
# Problem constants (hardcoded per contest rules; kernel.py is self-contained)
# ---------------------------------------------------------------------------
NUM_HEAD = 8
HIDDEN = 128  # head dim and final output dim
INPUT_DIM = 256
N1 = 1024
N2 = 1024
BATCH = 16
N_CORES = 8
BPC = BATCH // N_CORES  # batches per core
HD = NUM_HEAD * HIDDEN  # 1024
SDK = float(np.sqrt(np.float32(HIDDEN)))

F32 = mybir.dt.float32
F32R = mybir.dt.float32r
BF16 = mybir.dt.bfloat16
AF = mybir.ActivationFunctionType


# ---------------------------------------------------------------------------
# Post-pass: split multi-wait instructions into single-wait NOP prefixes.
# Walrus codegen in this container rejects instructions whose ISA struct has
# room for only one sync-wait command. A NoOp on the same engine queue
# carrying the extra waits is semantically identical (the sequencer executes
# waits in queue order before dispatching later instructions).
# ---------------------------------------------------------------------------
_ws_counter = [0]


def split_multi_waits(nc, cap=1):
    total = 0
    for fn in nc.m.functions:
        for blk in fn.blocks:
            insts = blk.instructions
            new = []
            changed = False
            for inst in insts:
                si = getattr(inst, "sync_info", None)
                waits = list(si.on_wait) if si is not None else []
                if len(waits) > cap:
                    for w in waits[:-cap]:
                        nop = mybir.InstNoOp(
                            name=f"I-wsplit-{_ws_counter[0]}", ins=[], outs=[]
                        )
                        _ws_counter[0] += 1
                        nop.engine = inst.engine
                        nop.sync_info = mybir.SyncInfo(on_wait=[w], on_update=[])
                        new.append(nop)
                        total += 1
                    inst.sync_info = mybir.SyncInfo(
                        on_wait=waits[-cap:], on_update=list(si.on_update)
                    )
                    changed = True
                new.append(inst)
            if changed:
                insts[:] = new
    return total


def _r(ap):
    """fp32 -> fp32r view for full-rate PE matmuls."""
    return ap.bitcast(F32R)


def build_bass(waitsplit=True, n_batches=BPC, n_heads=NUM_HEAD, do_attn=True):
    nc = bass.Bass()

    x1_d = nc.dram_tensor("x1", [BPC, N1, INPUT_DIM], F32, kind="ExternalInput")
    x2_d = nc.dram_tensor("x2", [BPC, N2, INPUT_DIM], F32, kind="ExternalInput")
    b_d = nc.dram_tensor("B", [N1, N2], F32, kind="ExternalInput")
    wq_d = nc.dram_tensor("wq_w", [HD, INPUT_DIM], F32, kind="ExternalInput")
    wk_d = nc.dram_tensor("wk_w", [HD, INPUT_DIM], F32, kind="ExternalInput")
    wv_d = nc.dram_tensor("wv_w", [HD, INPUT_DIM], F32, kind="ExternalInput")
    qb_d = nc.dram_tensor("wq_b", [HD], F32, kind="ExternalInput")
    kb_d = nc.dram_tensor("wk_b", [HD], F32, kind="ExternalInput")
    vb_d = nc.dram_tensor("wv_b", [HD], F32, kind="ExternalInput")
    pw_d = nc.dram_tensor("proj_w", [HIDDEN, HD], F32, kind="ExternalInput")
    pb_d = nc.dram_tensor("proj_b", [HIDDEN], F32, kind="ExternalInput")
    out_d = nc.dram_tensor("out", [BPC, N1, HIDDEN], F32, kind="ExternalOutput")

    NT1 = N1 // 128  # 8 n1 tiles
    NT2 = N2 // 128  # 8 n2 tiles
    CT = INPUT_DIM // 128  # 2 c tiles

    with tile.TileContext(nc) as tc:
        with (
            tc.tile_pool(name="const", bufs=1) as const,
            tc.tile_pool(name="psS", bufs=2, space="PSUM") as psS,
            tc.tile_pool(name="psO", bufs=1, space="PSUM") as psO,
            tc.tile_pool(name="psL", bufs=1, space="PSUM") as psL,
        ):
            ident = const.tile([128, 128], F32)
            make_identity(nc, ident)

            # weight/bias/B staging + one-time transposes (all bf16)
            wqT = const.tile([128, CT, HD], BF16)  # wq_w.T  [c, hd]
            wkT = const.tile([128, CT, HD], BF16)
            wvT = const.tile([128, CT, HD], BF16)
            projT = const.tile([128, NUM_HEAD, HIDDEN], BF16)  # proj_w.T [hd, o]
            eb = const.tile([128, NT2, N1], BF16)  # exp(B.T)  [n2, n1]
            qb_sb = const.tile([128, NUM_HEAD], F32)
            kb_sb = const.tile([128, NUM_HEAD], F32)
            pb_col = const.tile([128, 1], F32)
            ones128 = const.tile([128, 128], BF16)  # rowsum stationary
            vb_stage = const.tile([1, HD], F32)  # staged but unused (folded)

            nc.vector.memset(ones128, 1.0)
            nc.sync.dma_start(out=qb_sb, in_=qb_d.rearrange("(t p) -> p t", p=128))
            nc.sync.dma_start(out=kb_sb, in_=kb_d.rearrange("(t p) -> p t", p=128))
            nc.sync.dma_start(out=pb_col, in_=pb_d.rearrange("(p a) -> p a", a=1))
            nc.sync.dma_start(
                out=vb_stage, in_=vb_d.rearrange("(a n) -> a n", a=1)
            )

            with tc.tile_pool(name="stage", bufs=1) as stage:
                # --- weights: [hd, c] -> [c, hd] via PE transposes
                for w_d, wT in ((wq_d, wqT), (wk_d, wkT), (wv_d, wvT)):
                    wst = stage.tile([128, HD // 128, INPUT_DIM], F32, tag="wst")
                    nc.sync.dma_start(
                        out=wst, in_=w_d.rearrange("(t p) c -> p t c", p=128)
                    )
                    for ct in range(CT):
                        ps = psS.tile([128, 1024], F32, tag="s")
                        for t in range(HD // 128):
                            nc.tensor.transpose(
                                ps[:, t * 128 : (t + 1) * 128],
                                wst[:, t, ct * 128 : (ct + 1) * 128],
                                ident,
                            )
                        nc.scalar.copy(wT[:, ct, :], ps)
                # --- proj_w [o=128, hd] -> projT [hd, o]
                pwst = stage.tile([128, HD], F32)
                nc.sync.dma_start(out=pwst, in_=pw_d[:, :])
                ps = psS.tile([128, 1024], F32, tag="s")
                for h in range(NUM_HEAD):
                    nc.tensor.transpose(
                        ps[:, h * 128 : (h + 1) * 128],
                        pwst[:, h * 128 : (h + 1) * 128],
                        ident,
                    )
                nc.scalar.copy(projT.rearrange("p h o -> p (h o)"), ps)
                # --- B [n1, n2] -> eb = exp(B.T) [n2, n1] (bf16)
                bst = stage.tile([128, NT1, N2], F32)
                nc.sync.dma_start(
                    out=bst, in_=b_d.rearrange("(t p) m -> p t m", p=128)
                )
                for n2t in range(NT2):
                    ps = psS.tile([128, 1024], F32, tag="s")
                    for n1t in range(NT1):
                        nc.tensor.transpose(
                            ps[:, n1t * 128 : (n1t + 1) * 128],
                            bst[:, n1t, n2t * 128 : (n2t + 1) * 128],
                            ident,
                        )
                    nc.scalar.activation(eb[:, n2t, :], ps, AF.Exp)

            with (
                tc.tile_pool(name="xin", bufs=2) as xin,
                tc.tile_pool(name="qkv", bufs=1) as qkv,
                tc.tile_pool(name="attn", bufs=6) as attn,
                tc.tile_pool(name="head", bufs=2) as headp,
                tc.tile_pool(name="proj", bufs=2) as projp,
            ):
                for b in range(n_batches):
                    # ---------------- input transposes ----------------
                    x1T = xin.tile([128, CT, N1], BF16, tag="x1T")
                    x2T = xin.tile([128, CT, N2], BF16, tag="x2T")
                    for x_d, xT, nt in ((x1_d, x1T, NT1), (x2_d, x2T, NT2)):
                        xst = xin.tile([128, nt, INPUT_DIM], F32, tag="xst")
                        nc.sync.dma_start(
                            out=xst, in_=x_d[b].rearrange("(t p) c -> p t c", p=128)
                        )
                        for ct in range(CT):
                            ps = psS.tile([128, 1024], F32, tag="s")
                            for t in range(nt):
                                nc.tensor.transpose(
                                    ps[:, t * 128 : (t + 1) * 128],
                                    xst[:, t, ct * 128 : (ct + 1) * 128],
                                    ident,
                                )
                            nc.vector.tensor_copy(xT[:, ct, :], ps)

                    # ---------------- QKV projections ----------------
                    qT = qkv.tile([128, NUM_HEAD, N1], BF16, tag="qT")  # [d, n1]/head
                    kT = qkv.tile([128, NUM_HEAD, N2], BF16, tag="kT")  # [d, n2]/head
                    vN = qkv.tile([128, NT2, HD], BF16, tag="vN")  # [n2, hd]
                    # Q.T / K.T : out[hd_tile, n] ; lhsT = w.T slice, rhs = x.T
                    for xT, wT, dstT, bias, n in (
                        (x1T, wqT, qT, qb_sb, N1),
                        (x2T, wkT, kT, kb_sb, N2),
                    ):
                        for h in range(NUM_HEAD):
                            ps = psS.tile([128, 1024], F32, tag="s")
                            for half in range(2):
                                sl = slice(half * 512, half * 512 + 512)
                                for ct in range(CT):
                                    nc.tensor.matmul(
                                        ps[:, sl],
                                        wT[:, ct, h * 128 : (h + 1) * 128],
                                        xT[:, ct, sl],
                                        start=(ct == 0),
                                        stop=(ct == CT - 1),
                                    )
                            nc.scalar.activation(
                                dstT[:, h, :], ps, AF.Identity,
                                bias=bias[:, h : h + 1],
                            )
                    # V natural: out[n2_tile, hd] ; lhsT = x2.T slice, rhs = wv.T
                    for t in range(NT2):
                        ps = psS.tile([128, 1024], F32, tag="s")
                        for half in range(2):
                            sl = slice(half * 512, half * 512 + 512)
                            for ct in range(CT):
                                nc.tensor.matmul(
                                    ps[:, sl],
                                    x2T[:, ct, t * 128 : (t + 1) * 128],
                                    wvT[:, ct, sl],
                                    start=(ct == 0),
                                    stop=(ct == CT - 1),
                                )
                        nc.vector.tensor_copy(vN[:, t, :], ps)

                    # ---------------- attention + proj ----------------
                    # Each head's epilogue (normalize by 1/l, project into
                    # F.T) is deferred and emitted interleaved with the NEXT
                    # head's tile loop so the in-order PE queue never stalls
                    # on the DVE epilogue chain.
                    ftacc = projp.tile([128, N1], F32, tag="ft")  # F.T accum [o, n1]

                    def epi_recip(st):
                        st["linv"] = headp.tile([128, N1], F32, tag="linv", name="linv")
                        nc.vector.reciprocal_approx_fast(st["linv"], st["lrep"])

                    def epi_mul(st):
                        st["outT"] = headp.tile([128, N1], BF16, tag="outT", name="outT")
                        nc.vector.tensor_mul(st["outT"], st["po"], st["linv"])

                    def epi_proj(st):  # proj into F.T accumulation
                        h = st["h"]
                        fps = psS.tile([128, 1024], F32, tag="s")
                        for half in range(2):
                            sl = slice(half * 512, half * 512 + 512)
                            nc.tensor.matmul(
                                fps[:, sl], projT[:, h, :], st["outT"][:, sl],
                                start=True, stop=True,
                            )
                        if h == 0:
                            nc.vector.tensor_scalar_add(ftacc, fps, pb_col)
                        else:
                            nc.vector.tensor_add(ftacc, ftacc, fps)

                    pending = None
                    for h in range(n_heads if do_attn else 0):
                        po = psO.tile([128, N1], F32, tag="o")  # out_h.T accum
                        lrep = psL.tile([128, N1], F32, tag="l")  # replicated sums
                        for n2t in range(NT2):
                            sps = psS.tile([128, 1024], F32, tag="s")
                            p_t = attn.tile([128, N1], BF16, tag="p")
                            for half in range(2):
                                sl = slice(half * 512, half * 512 + 512)
                                nc.tensor.matmul(
                                    sps[:, sl],
                                    kT[:, h, n2t * 128 : (n2t + 1) * 128],
                                    qT[:, h, sl],
                                    start=True,
                                    stop=True,
                                )
                            # P = exp(S/sdk) * exp(B.T)
                            nc.scalar.activation(p_t, sps, AF.Exp, scale=1.0 / SDK)
                            nc.vector.tensor_mul(p_t, p_t, eb[:, n2t, :])
                            if pending:
                                if n2t == 0:
                                    epi_recip(pending)
                                elif n2t == 1:
                                    epi_mul(pending)
                                elif n2t == 2:
                                    epi_proj(pending)
                                    pending = None
                            first, last = n2t == 0, n2t == NT2 - 1
                            for half in range(2):
                                sl = slice(half * 512, half * 512 + 512)
                                nc.tensor.matmul(
                                    po[:, sl],
                                    vN[:, n2t, h * 128 : (h + 1) * 128],
                                    p_t[:, sl],
                                    start=first,
                                    stop=last,
                                    skip_group_check=True,
                                )
                                nc.tensor.matmul(
                                    lrep[:, sl],
                                    ones128,
                                    p_t[:, sl],
                                    start=first,
                                    stop=last,
                                    skip_group_check=True,
                                )
                        pending = {"h": h, "po": po, "lrep": lrep}
                    if pending:
                        epi_recip(pending)
                        epi_mul(pending)
                        epi_proj(pending)
                        pending = None

                    # ---------------- final transpose + store ----------------
                    if not do_attn or n_heads < 1:
                        nc.vector.memset(ftacc, 0.0)
                    ofin = projp.tile([128, NT1, HIDDEN], F32, tag="ofin")
                    for t4 in range(0, NT1, 4):
                        ps = psS.tile([128, 1024], F32, tag="s")
                        for j in range(4):
                            t = t4 + j
                            nc.tensor.transpose(
                                ps[:, j * 128 : (j + 1) * 128],
                                ftacc[:, t * 128 : (t + 1) * 128],
                                ident,
                            )
                        nc.scalar.copy(
                            ofin[:, t4 : t4 + 4, :].rearrange("p t o -> p (t o)"),
                            ps[:, 0:512],
                        )
                    nc.sync.dma_start(
                        out=out_d[b].rearrange("(t p) o -> p t o", p=128), in_=ofin
                    )

    # Populate .instr bytes for extended-inst InstISA subclasses (the
    # custom-DVE reciprocal) — Tile/raw-Bass skips this Bacc.compile() pass.
    from concourse.library_overlay import lower_extended_insts

    lower_extended_insts(nc)
    if waitsplit:
        split_multi_waits(nc)
    return nc


_NC_CACHE = {}


def _prep_shared(inputs):
    """Host-side input prep shared by kernel() and the test harness.

    wv_b is folded into proj_b: softmax rows sum to exactly 1, so
    attn @ (1 vb.T) = vb broadcast, and out @ proj_w.T picks up the
    constant proj_w @ vb.
    """
    shared = {
        n: np.ascontiguousarray(np.asarray(inputs[n], dtype=np.float32))
        for n in (
            "B", "wq_w", "wq_b", "wk_w", "wk_b", "wv_w", "wv_b", "proj_w", "proj_b"
        )
    }
    shared["proj_b"] = np.ascontiguousarray(
        shared["proj_b"] + shared["proj_w"] @ shared["wv_b"]
    )
    return shared


def kernel(**inputs) -> np.ndarray:
    from concourse.bass_utils import run_bass_kernel_spmd

    x1 = np.ascontiguousarray(np.asarray(inputs["x1"], dtype=np.float32))
    x2 = np.ascontiguousarray(np.asarray(inputs["x2"], dtype=np.float32))
    shared = _prep_shared(inputs)

    if "nc" not in _NC_CACHE:
        _NC_CACHE["nc"] = build_bass()
    nc = _NC_CACHE["nc"]

    in_maps = []
    for c in range(N_CORES):
        m = {"x1": x1[c * BPC : (c + 1) * BPC], "x2": x2[c * BPC : (c + 1) * BPC]}
        m.update(shared)
        in_maps.append(m)

    res = run_bass_kernel_spmd(nc, in_maps, core_ids=list(range(N_CORES)))
    out = np.concatenate([r["out"] for r in res.results], axis=0)
    return out
